# revision 1
# baseline (speedup 1.0000x reference)
"""Two-layer GRU (16->128->128) + FC(128->24) head on 8 Trainium2 NeuronCores.

Strategy: data-parallel over the batch (4096 -> 512 per core); tiny weights
replicated. On each core the hidden state lives transposed in SBUF as
[H=128 partitions, B=512 free]. Per time step, gate pre-activations are
accumulated in PSUM by fp32r matmuls (input-projection + recurrent + biases
folded in), sigmoid/tanh run on the scalar engine with per-partition bias
APs, and the cell update is spread across vector + gpsimd engines.

Self-contained: hardcodes all shapes; host-side prep only reshapes/transposes
numpy arrays (sharding + time-major packing of x, weight transposes).
"""

import numpy as np

import bass_rust
import concourse.bass as bass
import concourse.mybir as mybir
from concourse.tile import TileContext
from concourse.vector_clock import ScopedClock
from concourse.bass_utils import run_bass_kernel_spmd

N_CORES = 8
B_TOT = 4096
L = 128          # sequence length (= 2*1024/16)
D = 16           # per-step input features
DA = 17          # + ones row (bias folding for layer 1)
H = 128          # hidden
G3 = 3 * H       # 384 stacked gates (r, z, n)
BL = B_TOT // N_CORES  # 512 batch per core
NCLS = 24
CHUNK = 8        # time steps of x staged into SBUF per DMA

F32 = mybir.dt.float32
F32R = mybir.dt.float32r
F16 = mybir.dt.float16
BF16 = mybir.dt.bfloat16
AF = mybir.ActivationFunctionType
OP = mybir.AluOpType

# Tunables (grid-searched via TimelineSim, validated on HW).
CONFIG = {
    "dtype": "f16",      # gate/h/weight/x dtype: "f32r" | "f16" | "bf16"
    "pre_n_pe": True,    # accumulate t2 into P_x via PE identity matmul
    "split_rz1": True,  # separate r/z sigmoids for layer 1
    "d_eng": "v",        # engine for d = h - n
    "e_eng": "v",        # engine for e = z * d
    "h_eng": "v",        # engine for h' = n + e
}

_DT = {"f32r": F32R, "f16": F16, "bf16": BF16}
_NP_DT = {"f32r": np.float32, "f16": np.float16}


class SplitDrainTileContext(TileContext):
    """Walrus (CoreV3) rejects instructions carrying >2 sync waits; Tile's
    kernel-tail drain accumulates one wait per outstanding engine/DMA-queue
    sem. Split them across a chain of drains (1 wait each)."""

    def _drain_and_barrier(self, tick_clock, wait_clock):
        nc = self.nc
        drain_inst = nc.sync.drain()
        wait_clock.add_sem_waits(
            drain_inst.ins, ScopedClock({None: tick_clock.global_clock})
        )
        si = drain_inst.ins.sync_info
        if si is not None and len(si.on_wait) > 1:
            waits = list(si.on_wait)
            si.on_wait = waits[:1]
            for w in waits[1:]:
                d2 = nc.sync.drain()
                d2.ins.sync_info = bass_rust.SyncInfo(on_wait=[w], on_update=[])
        nc.all_engine_barrier()
        popped = nc._tile_sem_poison_stack.pop()
        assert popped is self._sem_poison
        nc.clear_and_free_semaphores(list(self.sems.allocated().values()))
        nc.all_engine_barrier()


def _split_excess_waits(nc: bass.Bass, max_waits: int = 1) -> None:
    """Walrus (CoreV3 setupSyncWait) accepts at most 2 sem waits per
    instruction; Tile occasionally attaches 3+. Hoist the excess onto
    EventSemaphore instructions inserted right before the offender on the
    same engine (serial waits AND together)."""
    n = 0
    for fn in nc.m.functions:
        for bb in fn.blocks:
            out = []
            dirty = False
            for inst in bb.instructions:
                si = inst.sync_info
                if si is not None and len(si.on_wait) > max_waits:
                    waits = list(si.on_wait)
                    extra = waits[: len(waits) - max_waits]
                    for w in extra:
                        ev = mybir.InstEventSemaphore(
                            name=f"evs-waitsplit-{n}", ins=[], outs=[]
                        )
                        n += 1
                        ev.engine = inst.engine
                        ev.sync_info = bass_rust.SyncInfo(
                            on_wait=[w], on_update=[]
                        )
                        out.append(ev)
                    si.on_wait = waits[len(waits) - max_waits :]
                    dirty = True
                out.append(inst)
            if dirty:
                bb.instructions = out


def build_program(for_sim: bool = False, n_steps: int = L) -> bass.Bass:
    # for_sim: skip the walrus wait-limit workarounds (post-hoc IR mutations
    # that CoreSim's bookkeeping doesn't understand); semantics identical.
    nc = bass.Bass()

    # Per-core DRAM I/O. Matmul operands are declared float32r (same bytes as
    # fp32) so the PE runs them at 1 cycle/row instead of fp32's 4.
    DT = _DT[CONFIG["dtype"]]
    xT_d = nc.declare_dram_parameter("xT", [L, DA, BL], DT, isOutput=False)
    l1w_d = nc.declare_dram_parameter("l1w", [DA, G3], DT, isOutput=False)
    hh1_d = nc.declare_dram_parameter("hh1w", [H, G3], DT, isOutput=False)
    ih2_d = nc.declare_dram_parameter("ih2w", [H, G3], DT, isOutput=False)
    hh2_d = nc.declare_dram_parameter("hh2w", [H, G3], DT, isOutput=False)
    bias_d = nc.declare_dram_parameter("bvec", [H, 5], F32, isOutput=False)
    fcw_d = nc.declare_dram_parameter("fcw", [H, NCLS], DT, isOutput=False)
    fcb_d = nc.declare_dram_parameter("fcb", [NCLS, 1], F32, isOutput=False)
    ident_d = nc.declare_dram_parameter("ident", [H, H], DT, isOutput=False)
    out_d = nc.declare_dram_parameter("outT", [NCLS, BL], F32, isOutput=True)

    tc_cls = TileContext if for_sim else SplitDrainTileContext
    with tc_cls(nc) as tc:
        with (
            tc.tile_pool(name="singles", bufs=1) as singles,
            tc.tile_pool(name="xchunks", bufs=3) as xpool,
            tc.tile_pool(name="hstate", bufs=2) as hpool,
            tc.tile_pool(name="work", bufs=3) as work,
            tc.tile_pool(name="prz", bufs=1, space="PSUM") as przpool,
            tc.tile_pool(name="pnx", bufs=1, space="PSUM") as pnxpool,
        ):
            # --- constant loads -------------------------------------------
            l1w = singles.tile([DA, G3], DT, tag="l1w")
            hh1w = singles.tile([H, G3], DT, tag="hh1w")
            ih2w = singles.tile([H, G3], DT, tag="ih2w")
            hh2w = singles.tile([H, G3], DT, tag="hh2w")
            sbias = singles.tile([H, 5], F32, tag="sbias")
            fcw = singles.tile([H, NCLS], DT, tag="fcw")
            fcb = singles.tile([NCLS, 1], F32, tag="fcb")
            ident = singles.tile([H, H], DT, tag="ident")
            nc.sync.dma_start(out=ident[:], in_=ident_d[:])
            nc.sync.dma_start(out=l1w[:], in_=l1w_d[:])
            nc.sync.dma_start(out=hh1w[:], in_=hh1_d[:])
            nc.sync.dma_start(out=ih2w[:], in_=ih2_d[:])
            nc.sync.dma_start(out=hh2w[:], in_=hh2_d[:])
            nc.sync.dma_start(out=sbias[:], in_=bias_d[:])
            nc.sync.dma_start(out=fcw[:], in_=fcw_d[:])
            nc.sync.dma_start(out=fcb[:], in_=fcb_d[:])

            ENG = {"v": nc.vector, "g": nc.gpsimd}

            def cell(tag, h_prev, x_rhs, xw, hw, rz_bias, n_hh_bias, n_ih_bias):
                """One GRU cell step, transposed layout [H partitions, BL free].

                h_prev: [H, BL] DT tile or None (t=0 => h=0, recurrent
                matmuls skipped). x_rhs: [K, BL] DT rhs for the input
                projection with lhsT xw [K, G3]; hw: [H, G3] recurrent lhsT.
                rz_bias: None (folded into xw) or (r_bias_ap, z_bias_ap).
                Returns the new [H, BL] DT hidden tile.
                """
                prz = przpool.tile([H, 2 * BL], F32, tag=f"prz{tag}")
                pn = pnxpool.tile([H, BL], F32, tag=f"pn{tag}")
                px = pnxpool.tile([H, BL], F32, tag=f"px{tag}")
                nc.tensor.matmul(prz[:, 0:BL], xw[:, 0:H], x_rhs,
                                 start=True, stop=h_prev is None)
                nc.tensor.matmul(prz[:, BL:], xw[:, H : 2 * H], x_rhs,
                                 start=True, stop=h_prev is None)
                if h_prev is not None:
                    nc.tensor.matmul(prz[:, 0:BL], hw[:, 0:H], h_prev[:],
                                     start=False, stop=True)
                    nc.tensor.matmul(prz[:, BL:], hw[:, H : 2 * H], h_prev[:],
                                     start=False, stop=True)
                nc.tensor.matmul(px[:], xw[:, 2 * H :], x_rhs, start=True,
                                 stop=not CONFIG["pre_n_pe"])
                if h_prev is not None:
                    nc.tensor.matmul(pn[:], hw[:, 2 * H :], h_prev[:],
                                     start=True, stop=True)

                split = rz_bias is not None or CONFIG["split_rz1"]
                if not split:
                    rz = work.tile([H, 2 * BL], DT, tag=f"rz{tag}")
                    nc.scalar.activation(rz[:], prz[:], AF.Sigmoid)
                    r, z = rz[:, 0:BL], rz[:, BL:]
                else:
                    rb = dict(bias=rz_bias[0]) if rz_bias else {}
                    zb = dict(bias=rz_bias[1]) if rz_bias else {}
                    rt = work.tile([H, BL], DT, tag=f"r{tag}")
                    nc.scalar.activation(rt[:], prz[:, 0:BL], AF.Sigmoid, **rb)
                    zt = work.tile([H, BL], DT, tag=f"z{tag}")
                    nc.scalar.activation(zt[:], prz[:, BL:], AF.Sigmoid, **zb)
                    r, z = rt[:], zt[:]

                t2 = work.tile([H, BL], DT, tag=f"t2{tag}")
                if h_prev is not None:
                    # t2 = (hn + b_hh_n) * r
                    nc.vector.scalar_tensor_tensor(
                        t2[:], pn[:], n_hh_bias, r, op0=OP.add, op1=OP.mult
                    )
                else:
                    nc.vector.tensor_scalar_mul(t2[:], r, n_hh_bias)
                n = work.tile([H, BL], DT, tag=f"n{tag}")
                nb = dict(bias=n_ih_bias) if n_ih_bias is not None else {}
                if CONFIG["pre_n_pe"]:
                    # px += I.T @ t2 on the PE, then tanh straight off PSUM
                    nc.tensor.matmul(px[:], ident[:], t2[:], start=False, stop=True)
                    nc.scalar.activation(n[:], px[:], AF.Tanh, **nb)
                else:
                    pre = work.tile([H, BL], F32, tag=f"pre{tag}")
                    nc.vector.tensor_add(pre[:], t2[:], px[:])
                    nc.scalar.activation(n[:], pre[:], AF.Tanh, **nb)
                d = work.tile([H, BL], DT, tag=f"d{tag}")
                if h_prev is not None:
                    ENG[CONFIG["d_eng"]].tensor_sub(d[:], h_prev[:], n[:])
                else:
                    ENG[CONFIG["d_eng"]].tensor_scalar_mul(d[:], n[:], -1.0)
                e = work.tile([H, BL], DT, tag=f"e{tag}")
                ENG[CONFIG["e_eng"]].tensor_mul(e[:], z, d[:])
                h_new = hpool.tile([H, BL], DT, tag=f"h{tag}")
                ENG[CONFIG["h_eng"]].tensor_add(h_new[:], n[:], e[:])
                return h_new

            h1 = None
            h2 = None
            xc = None
            for t in range(n_steps):
                if t % CHUNK == 0:
                    xc = xpool.tile([DA, CHUNK, BL], DT, tag="xc")
                    nc.sync.dma_start(
                        out=xc[:], in_=xT_d[t : t + CHUNK].rearrange("t d b -> d t b")
                    )
                xg = xc[:, t % CHUNK, :]
                h1 = cell("1", h1, xg, l1w, hh1w, None, sbias[:, 0:1], None)
                h2 = cell("2", h2, h1[:], ih2w, hh2w,
                          (sbias[:, 1:2], sbias[:, 2:3]), sbias[:, 3:4],
                          sbias[:, 4:5])

            # ---------------- FC head ------------------------------------
            pfc = pnxpool.tile([NCLS, BL], F32, tag="pn1")
            nc.tensor.matmul(pfc[:], fcw[:], h2[:], start=True, stop=True)
            outs = work.tile([NCLS, BL], F32, tag="outs")
            nc.scalar.activation(outs[:], pfc[:], AF.Identity, bias=fcb[:])
            nc.sync.dma_start(out=out_d[:], in_=outs[:])

    if not for_sim:
        _split_excess_waits(nc)
    return nc


def prep_in_maps(inputs: dict) -> list[dict]:
    """Shard + repack the full-problem numpy inputs into per-core in_maps."""
    x = np.ascontiguousarray(np.asarray(inputs["x"], dtype=np.float32))
    w_ih1 = np.asarray(inputs["w_ih1"], np.float32)
    w_hh1 = np.asarray(inputs["w_hh1"], np.float32)
    b_ih1 = np.asarray(inputs["b_ih1"], np.float32)
    b_hh1 = np.asarray(inputs["b_hh1"], np.float32)
    w_ih2 = np.asarray(inputs["w_ih2"], np.float32)
    w_hh2 = np.asarray(inputs["w_hh2"], np.float32)
    b_ih2 = np.asarray(inputs["b_ih2"], np.float32)
    b_hh2 = np.asarray(inputs["b_hh2"], np.float32)
    fc_w = np.asarray(inputs["fc_w"], np.float32)
    fc_b = np.asarray(inputs["fc_b"], np.float32)

    # x: (4096, 2, 1024) -> per-core time-major transposed [L, 17, BL]
    xr = x.reshape(N_CORES, BL, 2, L, D // 2)  # [core, b, ch, t, j]
    xT = np.empty((N_CORES, L, DA, BL), np.float32)
    xT[:, :, 0 : D // 2, :] = xr[:, :, 0].transpose(0, 2, 3, 1)
    xT[:, :, D // 2 : D, :] = xr[:, :, 1].transpose(0, 2, 3, 1)
    xT[:, :, D, :] = 1.0  # ones row: folds layer-1 biases into the matmul

    # layer-1 combined input-proj weights + bias row.
    # r/z columns carry b_ih1+b_hh1; n columns carry b_ih1 only (b_hh1_n must
    # be applied inside r*(hn+b_hh1_n)).
    l1w = np.empty((DA, G3), np.float32)
    l1w[0:D, :] = w_ih1.T
    bias_row = b_ih1.copy()
    bias_row[0 : 2 * H] += b_hh1[0 : 2 * H]
    l1w[D, :] = bias_row

    bvec = np.stack(
        [
            b_hh1[2 * H : 3 * H],                     # col 0: L1 n-gate hh bias
            (b_ih2 + b_hh2)[0:H],                     # col 1: L2 r bias
            (b_ih2 + b_hh2)[H : 2 * H],               # col 2: L2 z bias
            b_hh2[2 * H : 3 * H],                     # col 3: L2 n-gate hh bias
            b_ih2[2 * H : 3 * H],                     # col 4: L2 n-gate ih bias
        ],
        axis=1,
    ).astype(np.float32)

    if CONFIG["dtype"] == "bf16":
        import ml_dtypes
        ndt = np.dtype(ml_dtypes.bfloat16)
    else:
        ndt = _NP_DT[CONFIG["dtype"]]
    shared = {
        "l1w": np.ascontiguousarray(l1w).astype(ndt),
        "hh1w": np.ascontiguousarray(w_hh1.T).astype(ndt),
        "ih2w": np.ascontiguousarray(w_ih2.T).astype(ndt),
        "hh2w": np.ascontiguousarray(w_hh2.T).astype(ndt),
        "bvec": bvec,
        "fcw": np.ascontiguousarray(fc_w.T).astype(ndt),
        "fcb": np.ascontiguousarray(fc_b[:, None]),
        "ident": np.eye(H, dtype=np.float32).astype(ndt),
    }
    return [{"xT": np.ascontiguousarray(xT[c]).astype(ndt), **shared}
            for c in range(N_CORES)]


def assemble_output(results: list[dict]) -> np.ndarray:
    # per-core outT [24, BL] -> (4096, 24)
    return np.concatenate([r["outT"].T for r in results], axis=0).astype(np.float32)


_NC_CACHE = None


def kernel(**inputs) -> np.ndarray:
    global _NC_CACHE
    if _NC_CACHE is None:
        _NC_CACHE = build_program()
    in_maps = prep_in_maps(inputs)
    res = run_bass_kernel_spmd(_NC_CACHE, in_maps, list(range(N_CORES)))
    return assemble_output(res.results)



# revision 4
# speedup vs baseline: 38.0735x; 38.0735x over previous
"""Two-layer GRU (16->128->128) + FC(128->24) head on 8 Trainium2 NeuronCores.

Strategy: data-parallel over the batch (4096 -> 512 per core); tiny weights
replicated. On each core the hidden state lives transposed in SBUF as
[H=128 partitions, B=512 free]. Per time step, gate pre-activations are
accumulated in PSUM by f16 matmuls (input-projection + recurrent + biases
folded in), sigmoid/tanh run on the scalar engine with per-partition bias
APs, and the cell update is spread across vector + gpsimd engines.

The z-gate weights are negated host-side so sigmoid directly yields
zb = 1-z ("zbar" trick): h' = zb*n + (1-zb)*h = v + u with p = zb*h and
u = h - p computable before tanh(n) lands, leaving only two dependent ops
(v = zb*n, h' = v + u) after tanh on the recurrence-critical chain. The two
GRU layers are software-pipelined (L1 step t is emitted before L2 step t-1)
so L2's work fills engine idle slots of L1's serial chain. t2 = (pn+b)*r
stays on the vector engine (gpsimd cannot read PSUM).

build_program(repeat=R) wraps the whole forward pass in a hardware For_i
loop executing it R times back-to-back per dispatch; test.py times two
repeat counts and uses the slope to cancel multi-ms axon dispatch overhead.

Self-contained: hardcodes all shapes; host-side prep only reshapes/transposes
numpy arrays (sharding + time-major packing of x, weight transposes).
"""

import numpy as np

import bass_rust
import concourse.bass as bass
import concourse.mybir as mybir
from concourse.tile import TileContext
from concourse.vector_clock import ScopedClock
from concourse.bass_utils import run_bass_kernel_spmd

N_CORES = 8
B_TOT = 4096
L = 128          # sequence length (= 2*1024/16)
D = 16           # per-step input features
DA = 17          # + ones row (bias folding for layer 1)
H = 128          # hidden
G3 = 3 * H       # 384 stacked gates (r, z, n)
BL = B_TOT // N_CORES  # 512 batch per core
NCLS = 24
CHUNK = 8        # time steps of x staged into SBUF per DMA

F32 = mybir.dt.float32
F32R = mybir.dt.float32r
F16 = mybir.dt.float16
BF16 = mybir.dt.bfloat16
AF = mybir.ActivationFunctionType
OP = mybir.AluOpType

# Tunables (grid-searched via TimelineSim, validated on HW).
CONFIG = {
    "dtype": "f16",      # gate/h/weight/x dtype: "f32r" | "f16" | "bf16"
    "pre_n_pe": True,    # accumulate t2 into P_x via PE identity matmul
    "split_rz1": True,  # separate r/z sigmoids for layer 1
    "zbar": True,        # sigmoid yields 1-z (z-gate weights negated host-side)
    "fold2": False,      # L2 r/z biases via K=1 PE matmul; single combined sigmoid
    "l1ahead": True,     # emit L1 step t+1 before L2 step t (sw pipelining)
    # per-layer engine placement: v = DVE, g = gpsimd/Pool
    "p_eng1": "v", "p_eng2": "v",    # p = zb * h_prev (off-chain)
    "u_eng1": "v", "u_eng2": "v",    # u = h_prev - p  (off-chain)
    "v_eng1": "v", "v_eng2": "v",    # v = zb * n      (on-chain)
    "h_eng1": "v", "h_eng2": "v",    # h' = v + u      (on-chain)
    # NB: t2 reads pn from PSUM; gpsimd cannot access PSUM (walrus
    # birverifier rejects it), so t2 must stay on the vector engine.
    "t2_eng1": "v", "t2_eng2": "v",  # t2 = (pn+b)*r   (on-chain)
    "d_eng": "v",        # engine for d = h - n (legacy path)
    "e_eng": "v",        # engine for e = z * d (legacy path)
}

_DT = {"f32r": F32R, "f16": F16, "bf16": BF16}
_NP_DT = {"f32r": np.float32, "f16": np.float16}


class SplitDrainTileContext(TileContext):
    """Walrus (CoreV3) rejects instructions carrying >2 sync waits; Tile's
    kernel-tail drain accumulates one wait per outstanding engine/DMA-queue
    sem. Split them across a chain of drains (1 wait each)."""

    def _drain_and_barrier(self, tick_clock, wait_clock):
        nc = self.nc
        drain_inst = nc.sync.drain()
        wait_clock.add_sem_waits(
            drain_inst.ins, ScopedClock({None: tick_clock.global_clock})
        )
        si = drain_inst.ins.sync_info
        if si is not None and len(si.on_wait) > 1:
            waits = list(si.on_wait)
            si.on_wait = waits[:1]
            for w in waits[1:]:
                d2 = nc.sync.drain()
                d2.ins.sync_info = bass_rust.SyncInfo(on_wait=[w], on_update=[])
        nc.all_engine_barrier()
        popped = nc._tile_sem_poison_stack.pop()
        assert popped is self._sem_poison
        nc.clear_and_free_semaphores(list(self.sems.allocated().values()))
        nc.all_engine_barrier()


def _split_excess_waits(nc: bass.Bass, max_waits: int = 1) -> None:
    """Walrus (CoreV3 setupSyncWait) accepts at most 2 sem waits per
    instruction; Tile occasionally attaches 3+. Hoist the excess onto
    EventSemaphore instructions inserted right before the offender on the
    same engine (serial waits AND together)."""
    n = 0
    for fn in nc.m.functions:
        for bb in fn.blocks:
            out = []
            dirty = False
            for inst in bb.instructions:
                si = inst.sync_info
                if si is not None and len(si.on_wait) > max_waits:
                    waits = list(si.on_wait)
                    extra = waits[: len(waits) - max_waits]
                    for w in extra:
                        ev = mybir.InstEventSemaphore(
                            name=f"evs-waitsplit-{n}", ins=[], outs=[]
                        )
                        n += 1
                        ev.engine = inst.engine
                        ev.sync_info = bass_rust.SyncInfo(
                            on_wait=[w], on_update=[]
                        )
                        out.append(ev)
                    si.on_wait = waits[len(waits) - max_waits :]
                    dirty = True
                out.append(inst)
            if dirty:
                bb.instructions = out


def build_program(for_sim: bool = False, n_steps: int = L,
                  repeat: int = 1) -> bass.Bass:
    # for_sim: skip the walrus wait-limit workarounds (post-hoc IR mutations
    # that CoreSim's bookkeeping doesn't understand); semantics identical.
    # repeat: wrap the whole forward pass in a hardware For_i loop that runs
    # it `repeat` times back-to-back (used by test.py to measure marginal
    # per-execution HW time without per-dispatch overhead).
    nc = bass.Bass()

    # Per-core DRAM I/O. Matmul operands are declared float32r (same bytes as
    # fp32) so the PE runs them at 1 cycle/row instead of fp32's 4.
    DT = _DT[CONFIG["dtype"]]
    xT_d = nc.declare_dram_parameter("xT", [L, DA, BL], DT, isOutput=False)
    l1w_d = nc.declare_dram_parameter("l1w", [DA, G3], DT, isOutput=False)
    hh1_d = nc.declare_dram_parameter("hh1w", [H, G3], DT, isOutput=False)
    ih2_d = nc.declare_dram_parameter("ih2w", [H, G3], DT, isOutput=False)
    hh2_d = nc.declare_dram_parameter("hh2w", [H, G3], DT, isOutput=False)
    bias_d = nc.declare_dram_parameter("bvec", [H, 5], F32, isOutput=False)
    brz2_d = (nc.declare_dram_parameter("brz2T", [1, 2 * H], DT, isOutput=False)
              if CONFIG["fold2"] else None)
    fcw_d = nc.declare_dram_parameter("fcw", [H, NCLS], DT, isOutput=False)
    fcb_d = nc.declare_dram_parameter("fcb", [NCLS, 1], F32, isOutput=False)
    ident_d = nc.declare_dram_parameter("ident", [H, H], DT, isOutput=False)
    out_d = nc.declare_dram_parameter("outT", [NCLS, BL], F32, isOutput=True)

    tc_cls = TileContext if for_sim else SplitDrainTileContext
    with tc_cls(nc) as tc:
        with (
            tc.tile_pool(name="singles", bufs=1) as singles,
            tc.tile_pool(name="xchunks", bufs=3) as xpool,
            tc.tile_pool(name="hstate", bufs=4) as hpool,
            tc.tile_pool(name="work", bufs=3) as work,
            tc.tile_pool(name="prz", bufs=1, space="PSUM") as przpool,
            tc.tile_pool(name="pnx", bufs=1, space="PSUM") as pnxpool,
        ):
            # --- constant loads -------------------------------------------
            l1w = singles.tile([DA, G3], DT, tag="l1w")
            hh1w = singles.tile([H, G3], DT, tag="hh1w")
            ih2w = singles.tile([H, G3], DT, tag="ih2w")
            hh2w = singles.tile([H, G3], DT, tag="hh2w")
            sbias = singles.tile([H, 5], F32, tag="sbias")
            fcw = singles.tile([H, NCLS], DT, tag="fcw")
            fcb = singles.tile([NCLS, 1], F32, tag="fcb")
            ident = singles.tile([H, H], DT, tag="ident")
            if CONFIG["fold2"]:
                brz2 = singles.tile([1, 2 * H], DT, tag="brz2")
                ones = singles.tile([1, BL], DT, tag="ones")
                nc.vector.memset(ones[:], 1.0)
                nc.sync.dma_start(out=brz2[:], in_=brz2_d[:])
            nc.sync.dma_start(out=ident[:], in_=ident_d[:])
            nc.sync.dma_start(out=l1w[:], in_=l1w_d[:])
            nc.sync.dma_start(out=hh1w[:], in_=hh1_d[:])
            nc.sync.dma_start(out=ih2w[:], in_=ih2_d[:])
            nc.sync.dma_start(out=hh2w[:], in_=hh2_d[:])
            nc.sync.dma_start(out=sbias[:], in_=bias_d[:])
            nc.sync.dma_start(out=fcw[:], in_=fcw_d[:])
            nc.sync.dma_start(out=fcb[:], in_=fcb_d[:])

            ENG = {"v": nc.vector, "g": nc.gpsimd}

            def cell(tag, h_prev, x_rhs, xw, hw, rz_bias, n_hh_bias, n_ih_bias,
                     fold_rz=None):
                """One GRU cell step, transposed layout [H partitions, BL free].

                h_prev: [H, BL] DT tile or None (t=0 => h=0, recurrent
                matmuls skipped). x_rhs: [K, BL] DT rhs for the input
                projection with lhsT xw [K, G3]; hw: [H, G3] recurrent lhsT.
                rz_bias: None (folded into xw) or (r_bias_ap, z_bias_ap).
                Returns the new [H, BL] DT hidden tile.
                """
                prz = przpool.tile([H, 2 * BL], F32, tag=f"prz{tag}")
                pn = pnxpool.tile([H, BL], F32, tag=f"pn{tag}")
                px = pnxpool.tile([H, BL], F32, tag=f"px{tag}")
                if fold_rz is not None:
                    # r/z biases enter PSUM first via K=1 rank-1 matmuls
                    # (bias_row x ones_row), off the critical chain.
                    nc.tensor.matmul(prz[:, 0:BL], brz2[:, 0:H], ones[:],
                                     start=True, stop=False)
                    nc.tensor.matmul(prz[:, BL:], brz2[:, H :], ones[:],
                                     start=True, stop=False)
                nc.tensor.matmul(prz[:, 0:BL], xw[:, 0:H], x_rhs,
                                 start=fold_rz is None, stop=h_prev is None)
                nc.tensor.matmul(prz[:, BL:], xw[:, H : 2 * H], x_rhs,
                                 start=fold_rz is None, stop=h_prev is None)
                if h_prev is not None:
                    nc.tensor.matmul(prz[:, 0:BL], hw[:, 0:H], h_prev[:],
                                     start=False, stop=True)
                    nc.tensor.matmul(prz[:, BL:], hw[:, H : 2 * H], h_prev[:],
                                     start=False, stop=True)
                nc.tensor.matmul(px[:], xw[:, 2 * H :], x_rhs, start=True,
                                 stop=not CONFIG["pre_n_pe"])
                if h_prev is not None:
                    nc.tensor.matmul(pn[:], hw[:, 2 * H :], h_prev[:],
                                     start=True, stop=True)

                split = (rz_bias is not None or CONFIG["split_rz1"]) and \
                    fold_rz is None
                if not split:
                    rz = work.tile([H, 2 * BL], DT, tag=f"rz{tag}")
                    nc.scalar.activation(rz[:], prz[:], AF.Sigmoid)
                    r, z = rz[:, 0:BL], rz[:, BL:]
                else:
                    rb = dict(bias=rz_bias[0]) if rz_bias else {}
                    zb = dict(bias=rz_bias[1]) if rz_bias else {}
                    rt = work.tile([H, BL], DT, tag=f"r{tag}")
                    nc.scalar.activation(rt[:], prz[:, 0:BL], AF.Sigmoid, **rb)
                    zt = work.tile([H, BL], DT, tag=f"z{tag}")
                    nc.scalar.activation(zt[:], prz[:, BL:], AF.Sigmoid, **zb)
                    r, z = rt[:], zt[:]

                if CONFIG["zbar"] and h_prev is not None:
                    # z tile actually holds zb = 1-z (z-gate weights negated
                    # host-side). h' = zb*n + z*h = v + u with
                    # p = zb*h, u = h - p, v = zb*n; p/u run off the critical
                    # chain while t2/tanh are still in flight.
                    p = work.tile([H, BL], DT, tag=f"p{tag}")
                    ENG[CONFIG[f"p_eng{tag}"]].tensor_mul(p[:], z, h_prev[:])
                    u = work.tile([H, BL], DT, tag=f"u{tag}")
                    ENG[CONFIG[f"u_eng{tag}"]].tensor_sub(u[:], h_prev[:], p[:])

                t2 = work.tile([H, BL], DT, tag=f"t2{tag}")
                if h_prev is not None:
                    # t2 = (hn + b_hh_n) * r
                    ENG[CONFIG[f"t2_eng{tag}"]].scalar_tensor_tensor(
                        t2[:], pn[:], n_hh_bias, r, op0=OP.add, op1=OP.mult
                    )
                else:
                    nc.vector.tensor_scalar_mul(t2[:], r, n_hh_bias)
                n = work.tile([H, BL], DT, tag=f"n{tag}")
                nb = dict(bias=n_ih_bias) if n_ih_bias is not None else {}
                if CONFIG["pre_n_pe"]:
                    # px += I.T @ t2 on the PE, then tanh straight off PSUM
                    nc.tensor.matmul(px[:], ident[:], t2[:], start=False, stop=True)
                    nc.scalar.activation(n[:], px[:], AF.Tanh, **nb)
                else:
                    pre = work.tile([H, BL], F32, tag=f"pre{tag}")
                    nc.vector.tensor_add(pre[:], t2[:], px[:])
                    nc.scalar.activation(n[:], pre[:], AF.Tanh, **nb)
                h_new = hpool.tile([H, BL], DT, tag=f"h{tag}")
                if CONFIG["zbar"]:
                    if h_prev is None:
                        # h' = zb * n at t=0
                        ENG[CONFIG[f"h_eng{tag}"]].tensor_mul(h_new[:], z, n[:])
                    else:
                        v = work.tile([H, BL], DT, tag=f"v{tag}")
                        ENG[CONFIG[f"v_eng{tag}"]].tensor_mul(v[:], z, n[:])
                        ENG[CONFIG[f"h_eng{tag}"]].tensor_add(h_new[:], v[:], u[:])
                else:
                    d = work.tile([H, BL], DT, tag=f"d{tag}")
                    if h_prev is not None:
                        ENG[CONFIG["d_eng"]].tensor_sub(d[:], h_prev[:], n[:])
                    else:
                        ENG[CONFIG["d_eng"]].tensor_scalar_mul(d[:], n[:], -1.0)
                    e = work.tile([H, BL], DT, tag=f"e{tag}")
                    ENG[CONFIG["e_eng"]].tensor_mul(e[:], z, d[:])
                    ENG[CONFIG[f"h_eng{tag}"]].tensor_add(h_new[:], n[:], e[:])
                return h_new

            def body():
                h1 = None
                h2 = None
                xc = None
                if CONFIG["fold2"]:
                    l2args = (ih2w, hh2w, None, sbias[:, 3:4], sbias[:, 4:5])
                    l2kw = dict(fold_rz=True)
                else:
                    l2args = (ih2w, hh2w, (sbias[:, 1:2], sbias[:, 2:3]),
                              sbias[:, 3:4], sbias[:, 4:5])
                    l2kw = {}
                if CONFIG["l1ahead"]:
                    # Software pipeline: issue L1 step t, then L2 step t-1,
                    # so L2's engine work never sits ahead of L1's recurrence
                    # chain in the in-order queues.
                    h1prev = None
                    for t in range(n_steps):
                        if t % CHUNK == 0:
                            xc = xpool.tile([DA, CHUNK, BL], DT, tag="xc")
                            nc.sync.dma_start(
                                out=xc[:],
                                in_=xT_d[t : t + CHUNK].rearrange(
                                    "t d b -> d t b"),
                            )
                        xg = xc[:, t % CHUNK, :]
                        h1, h1prev = (
                            cell("1", h1, xg, l1w, hh1w, None, sbias[:, 0:1],
                                 None),
                            h1,
                        )
                        if t >= 1:
                            h2 = cell("2", h2, h1prev[:], *l2args, **l2kw)
                    h2 = cell("2", h2, h1[:], *l2args, **l2kw)
                else:
                    for t in range(n_steps):
                        if t % CHUNK == 0:
                            xc = xpool.tile([DA, CHUNK, BL], DT, tag="xc")
                            nc.sync.dma_start(
                                out=xc[:],
                                in_=xT_d[t : t + CHUNK].rearrange(
                                    "t d b -> d t b"),
                            )
                        xg = xc[:, t % CHUNK, :]
                        h1 = cell("1", h1, xg, l1w, hh1w, None, sbias[:, 0:1],
                                  None)
                        h2 = cell("2", h2, h1[:], *l2args, **l2kw)

                # ---------------- FC head --------------------------------
                pfc = pnxpool.tile([NCLS, BL], F32, tag="pn1")
                nc.tensor.matmul(pfc[:], fcw[:], h2[:], start=True, stop=True)
                outs = work.tile([NCLS, BL], F32, tag="outs")
                nc.scalar.activation(outs[:], pfc[:], AF.Identity,
                                     bias=fcb[:])
                nc.sync.dma_start(out=out_d[:], in_=outs[:])

            if repeat == 1:
                body()
            else:
                with tc.For_i(0, repeat):
                    body()

    if not for_sim:
        _split_excess_waits(nc)
    return nc


def prep_in_maps(inputs: dict) -> list[dict]:
    """Shard + repack the full-problem numpy inputs into per-core in_maps."""
    x = np.ascontiguousarray(np.asarray(inputs["x"], dtype=np.float32))
    w_ih1 = np.asarray(inputs["w_ih1"], np.float32)
    w_hh1 = np.asarray(inputs["w_hh1"], np.float32)
    b_ih1 = np.asarray(inputs["b_ih1"], np.float32)
    b_hh1 = np.asarray(inputs["b_hh1"], np.float32)
    w_ih2 = np.asarray(inputs["w_ih2"], np.float32)
    w_hh2 = np.asarray(inputs["w_hh2"], np.float32)
    b_ih2 = np.asarray(inputs["b_ih2"], np.float32)
    b_hh2 = np.asarray(inputs["b_hh2"], np.float32)
    fc_w = np.asarray(inputs["fc_w"], np.float32)
    fc_b = np.asarray(inputs["fc_b"], np.float32)

    # x: (4096, 2, 1024) -> per-core time-major transposed [L, 17, BL]
    xr = x.reshape(N_CORES, BL, 2, L, D // 2)  # [core, b, ch, t, j]
    xT = np.empty((N_CORES, L, DA, BL), np.float32)
    xT[:, :, 0 : D // 2, :] = xr[:, :, 0].transpose(0, 2, 3, 1)
    xT[:, :, D // 2 : D, :] = xr[:, :, 1].transpose(0, 2, 3, 1)
    xT[:, :, D, :] = 1.0  # ones row: folds layer-1 biases into the matmul

    # layer-1 combined input-proj weights + bias row.
    # r/z columns carry b_ih1+b_hh1; n columns carry b_ih1 only (b_hh1_n must
    # be applied inside r*(hn+b_hh1_n)).
    l1w = np.empty((DA, G3), np.float32)
    l1w[0:D, :] = w_ih1.T
    bias_row = b_ih1.copy()
    bias_row[0 : 2 * H] += b_hh1[0 : 2 * H]
    l1w[D, :] = bias_row

    bvec = np.stack(
        [
            b_hh1[2 * H : 3 * H],                     # col 0: L1 n-gate hh bias
            (b_ih2 + b_hh2)[0:H],                     # col 1: L2 r bias
            (b_ih2 + b_hh2)[H : 2 * H],               # col 2: L2 z bias
            b_hh2[2 * H : 3 * H],                     # col 3: L2 n-gate hh bias
            b_ih2[2 * H : 3 * H],                     # col 4: L2 n-gate ih bias
        ],
        axis=1,
    ).astype(np.float32)

    brz2T = np.concatenate([(b_ih2 + b_hh2)[0:H], (b_ih2 + b_hh2)[H : 2 * H]]
                           )[None, :]
    hh1w_t = w_hh1.T.copy()
    ih2w_t = w_ih2.T.copy()
    hh2w_t = w_hh2.T.copy()
    if CONFIG["zbar"]:
        # Negate every z-gate pre-activation so sigmoid yields zb = 1-z.
        l1w[:, H : 2 * H] *= -1.0
        hh1w_t[:, H : 2 * H] *= -1.0
        ih2w_t[:, H : 2 * H] *= -1.0
        hh2w_t[:, H : 2 * H] *= -1.0
        bvec[:, 2] *= -1.0
        brz2T = brz2T.copy()
        brz2T[0, H:] *= -1.0

    if CONFIG["dtype"] == "bf16":
        import ml_dtypes
        ndt = np.dtype(ml_dtypes.bfloat16)
    else:
        ndt = _NP_DT[CONFIG["dtype"]]
    shared = {
        "l1w": np.ascontiguousarray(l1w).astype(ndt),
        "hh1w": np.ascontiguousarray(hh1w_t).astype(ndt),
        "ih2w": np.ascontiguousarray(ih2w_t).astype(ndt),
        "hh2w": np.ascontiguousarray(hh2w_t).astype(ndt),
        "bvec": bvec,
        **({"brz2T": np.ascontiguousarray(brz2T).astype(ndt)}
           if CONFIG["fold2"] else {}),
        "fcw": np.ascontiguousarray(fc_w.T).astype(ndt),
        "fcb": np.ascontiguousarray(fc_b[:, None]),
        "ident": np.eye(H, dtype=np.float32).astype(ndt),
    }
    return [{"xT": np.ascontiguousarray(xT[c]).astype(ndt), **shared}
            for c in range(N_CORES)]


def assemble_output(results: list[dict]) -> np.ndarray:
    # per-core outT [24, BL] -> (4096, 24)
    return np.concatenate([r["outT"].T for r in results], axis=0).astype(np.float32)


_NC_CACHE = None


def kernel(**inputs) -> np.ndarray:
    global _NC_CACHE
    if _NC_CACHE is None:
        _NC_CACHE = build_program()
    in_maps = prep_in_maps(inputs)
    res = run_bass_kernel_spmd(_NC_CACHE, in_maps, list(range(N_CORES)))
    return assemble_output(res.results)

